# revision 1
# baseline (speedup 1.0000x reference)
"""Conditional (per-row expert) linear layer for Trainium2, 8 NeuronCores.

Math: out[i] = W[c_i] @ x[i] + sum_c b[c]    (x: [B,D], W: [C,D,D], b: [C,D])

Strategy: expert-parallel. Core c handles exactly the rows with
condition_ids == c (gathered on host, padded to a common capacity so the
SPMD NEFF has static shapes). Each core runs one [n_cap, D] @ [D, D] GEMM
in float32r (fp32 bytes, FP22 multiply on the PE) plus a broadcast bias
add, then the host scatters rows back. This does 1/C of the reference's
dense FLOPs and reads only its own expert's weights.

Measured on the 8-core axon TRN2 (steady-state per-execution, rep-slope
method): ~40us; PE floor for the 144 N=512 matmuls is ~37.6us
(~261ns/MM regardless of dtype), DMA ~12.7MB/core/exec split across
HWDGE (W, half the outputs) and 4 SWDGE queues (x, other outputs).
Accuracy vs fp64 oracle: 1.25e-4 rms rel (float32r FP22 truncation).
"""

import sys
from contextlib import ExitStack

import numpy as np

try:
    import concourse.bass as bass  # noqa: F401
except ImportError:  # pragma: no cover
    sys.path.insert(0, "/opt/trn_rl_repo")

import jax
from jax.experimental.shard_map import shard_map
from jax.sharding import Mesh, PartitionSpec

import concourse.mybir as mybir
import concourse.tile as tile
from concourse import bacc
from concourse import bass2jax as _b2j

B, D, C = 8192, 1024, 8
P = 128  # partitions
KT = D // P  # k-tiles along the contraction dim
HALF = 512  # PSUM half-bank free size (fp32)

_cache: dict[int, "_Runner"] = {}


def _build(n_cap: int, reps: int = 1):
    """Per-core program: out[n, o] = xT.T @ WT + bias, n_cap x D output.

    reps > 1 repeats the whole body (including all DMAs) back-to-back for
    benchmarking: wall(T) - wall(1) isolates per-execution device time."""
    assert n_cap % 32 == 0
    row_tiles = [(s, min(P, n_cap - s)) for s in range(0, n_cap, P)]
    nc = bacc.Bacc("TRN2", target_bir_lowering=False, debug=False, num_devices=8, num_swdge_queues=4)
    xT = nc.dram_tensor("xT", [D, n_cap], mybir.dt.float32r, kind="ExternalInput").ap()
    WT = nc.dram_tensor("WT", [D, D], mybir.dt.float32r, kind="ExternalInput").ap()
    bias = nc.dram_tensor("bias", [1, D], mybir.dt.float32, kind="ExternalInput").ap()
    out = nc.dram_tensor("out", [n_cap, D], mybir.dt.float32, kind="ExternalOutput").ap()

    with tile.TileContext(nc) as tc, ExitStack() as ctx:
        w_pool = ctx.enter_context(tc.tile_pool(name="w", bufs=2))
        x_pool = ctx.enter_context(tc.tile_pool(name="x", bufs=2))
        b_pool = ctx.enter_context(tc.tile_pool(name="b", bufs=1))
        o_pool = ctx.enter_context(tc.tile_pool(name="o", bufs=1))
        ps_pool = ctx.enter_context(tc.tile_pool(name="ps", bufs=4, space="PSUM"))

        bias_row = b_pool.tile([1, D], mybir.dt.float32, name="bias_row", tag="bias_row")
        nc.sync.dma_start(bias_row[:], bias[:])
        bias_sb = b_pool.tile([P, D], mybir.dt.float32, name="bias_sb", tag="bias_sb")
        nc.gpsimd.partition_broadcast(bias_sb[:, :], bias_row[0:1, :])

        # Two k-groups with separate PSUM accumulations, combined on DVE.
        # A row-tile's group-A matmuls only need k0..k3 in SBUF, so the PE
        # starts long before the full input fill lands — the single-shot
        # fill (~9MB) overlaps the PE work instead of serializing it.
        k_groups = [range(0, KT // 2), range(KT // 2, KT)]
        xh = n_cap // 64 * 32  # x column split point (row dim), 32-aligned

        for _rep in range(reps):
            w_tiles, x_tiles = [], []
            for k in range(KT):
                wt = w_pool.tile([P, D], mybir.dt.float32r, name=f"wt{k}", tag=f"wt{k}")
                nc.sync.dma_start(wt[:, 0:HALF], WT[k * P : (k + 1) * P, 0:HALF])
                nc.sync.dma_start(wt[:, HALF:D], WT[k * P : (k + 1) * P, HALF:D])
                xt = x_pool.tile(
                    [P, n_cap], mybir.dt.float32r, name=f"xt{k}", tag=f"xt{k}"
                )
                nc.gpsimd.dma_start(xt[:, 0:xh], xT[k * P : (k + 1) * P, 0:xh])
                nc.gpsimd.dma_start(xt[:, xh:n_cap], xT[k * P : (k + 1) * P, xh:n_cap])
                w_tiles.append(wt)
                x_tiles.append(xt)

            o_tiles = {}
            for gi, ks in enumerate(k_groups):
                for start, size in row_tiles:
                    ps = ps_pool.tile([P, D], mybir.dt.float32, name="ps", tag="ps")
                    for k in ks:
                        for lo in (0, HALF):
                            nc.tensor.matmul(
                                ps[:size, lo : lo + HALF],
                                x_tiles[k][:, start : start + size],
                                w_tiles[k][:, lo : lo + HALF],
                                start=(k == ks[0]),
                                stop=(k == ks[-1]),
                                skip_group_check=True,
                            )
                    if gi == 0:
                        o_sb = o_pool.tile(
                            [P, D], mybir.dt.float32, name=f"o{start}", tag=f"o{start}"
                        )
                        o_tiles[start] = o_sb
                        nc.vector.tensor_add(
                            o_sb[:size, :], ps[:size, :], bias_sb[:size, :]
                        )
                    else:
                        o_sb = o_tiles[start]
                        nc.vector.tensor_add(o_sb[:size, :], o_sb[:size, :], ps[:size, :])
                    if gi == len(k_groups) - 1:
                        out_eng = nc.sync if (start // P) % 2 == 0 else nc.gpsimd
                        out_eng.dma_start(out[start : start + size, :], o_sb[:size, :])

    nc.compile()
    _check_noload_pairs(nc)
    return nc


def _check_noload_pairs(nc):
    """Every ldweights=False matmul must immediately follow (in PE stream
    order) a matmul with the identical stationary AP — otherwise the PE
    array would hold the wrong weights. Scheduling is deterministic at
    build time, so passing here guarantees correctness on device."""
    prev_mm = None
    for fn in nc.m.functions:
        for blk in fn.blocks:
            for inst in blk.instructions:
                if type(inst).__name__ != "InstMatmult":
                    continue
                if inst.ldweights is False:
                    assert prev_mm is not None, "no-load matmul with no predecessor"
                    same = str(prev_mm.ins[1]) == str(inst.ins[1])
                    assert same, (
                        f"no-load matmul stationary mismatch:\n"
                        f"prev: {prev_mm.ins[1]}\nthis: {inst.ins[1]}"
                    )
                prev_mm = inst


class _Runner:
    """Caches the compiled NEFF + jitted shard_map executable for one n_cap."""

    def __init__(self, n_cap: int, reps: int = 1):
        self.n_cap = n_cap
        self.nc = _build(n_cap, reps)
        _b2j.install_neuronx_cc_hook()

        assert self.nc.dbg_addr is None
        partition_name = (
            self.nc.partition_id_tensor.name if self.nc.partition_id_tensor else None
        )

        in_names, out_names, out_avals = [], [], []
        for alloc in self.nc.m.functions[0].allocations:
            if not isinstance(alloc, mybir.MemoryLocationSet):
                continue
            name = alloc.memorylocations[0].name
            if alloc.kind == "ExternalInput":
                if name != partition_name:
                    in_names.append(name)
            elif alloc.kind == "ExternalOutput":
                out_names.append(name)
                out_avals.append(
                    jax.core.ShapedArray(
                        tuple(alloc.tensor_shape), mybir.dt.np(alloc.dtype)
                    )
                )
        self.in_names = in_names
        self.out_names = out_names
        self.out_avals = out_avals
        self.n_params = len(in_names)
        self.n_outs = len(out_names)
        all_in_names = tuple(in_names + out_names)
        if partition_name is not None:
            all_in_names = all_in_names + (partition_name,)

        nc = self.nc

        def _bind(*args):
            operands = list(args)
            if partition_name is not None:
                operands.append(_b2j.partition_id_tensor())
            return tuple(
                _b2j._bass_exec_p.bind(
                    *operands,
                    out_avals=tuple(out_avals),
                    in_names=all_in_names,
                    out_names=tuple(out_names),
                    lowering_input_output_aliases=(),
                    sim_require_finite=True,
                    sim_require_nnan=True,
                    nc=nc,
                )
            )

        self._bind = _bind
        self.devices = jax.devices("neuron")[:C]
        self.mesh = Mesh(np.asarray(self.devices), ("core",))
        spec_in = (PartitionSpec("core"),) * (self.n_params + self.n_outs)
        spec_out = (PartitionSpec("core"),) * self.n_outs
        self._spec_in, self._spec_out = spec_in, spec_out
        self._exec = jax.jit(
            shard_map(
                _bind,
                mesh=self.mesh,
                in_specs=spec_in,
                out_specs=spec_out,
                check_rep=False,
            ),
            donate_argnums=tuple(range(self.n_params, self.n_params + self.n_outs)),
            keep_unused=True,
        )

    def make_exec_nodonate(self):
        """Jitted executable that does not donate its output-init operands,
        so pre-staged device args can be reused across timing reps."""
        return jax.jit(
            shard_map(
                self._bind,
                mesh=self.mesh,
                in_specs=self._spec_in,
                out_specs=self._spec_out,
                check_rep=False,
            ),
            keep_unused=True,
        )

    def concat_inputs(self, in_maps):
        return [
            np.concatenate([np.asarray(m[name]) for m in in_maps], axis=0)
            for name in self.in_names
        ]

    def zero_outs(self):
        return [
            np.zeros((C * a.shape[0], *a.shape[1:]), a.dtype) for a in self.out_avals
        ]

    def run(self, in_maps):
        out_arrs = self._exec(*self.concat_inputs(in_maps), *self.zero_outs())
        return [
            {
                name: np.asarray(out_arrs[i]).reshape(C, *self.out_avals[i].shape)[c]
                for i, name in enumerate(self.out_names)
            }
            for c in range(C)
        ]


def _get(n_cap: int, reps: int = 1) -> _Runner:
    key = (n_cap, reps)
    if key not in _cache:
        _cache[key] = _Runner(n_cap, reps)
    return _cache[key]


def _prep(x, condition_ids, W, b):
    x = np.ascontiguousarray(np.asarray(x, dtype=np.float32))
    cond = np.asarray(condition_ids).astype(np.int64)
    W = np.asarray(W, dtype=np.float32)
    b = np.asarray(b, dtype=np.float32)

    bias_tile = np.ascontiguousarray(b.sum(axis=0, dtype=np.float32)[None, :])

    rows = [np.nonzero(cond == c)[0] for c in range(C)]
    n_max = max(len(r) for r in rows)
    n_cap = max(32, -(-n_max // 32) * 32)

    in_maps = []
    for c in range(C):
        r = rows[c]
        xg = np.zeros((n_cap, D), np.float32)
        xg[: len(r)] = x[r]
        in_maps.append(
            {
                "xT": np.ascontiguousarray(xg.T),
                "WT": np.ascontiguousarray(W[c].T),
                "bias": bias_tile,
            }
        )
    return rows, n_cap, in_maps


def _run(x, condition_ids, W, b, trace=False):
    rows, n_cap, in_maps = _prep(x, condition_ids, W, b)
    runner = _get(n_cap)
    results = runner.run(in_maps)

    out = np.empty((B, D), np.float32)
    for c in range(C):
        r = rows[c]
        out[r] = results[c]["out"][: len(r)]
    return out, runner


def kernel(x, condition_ids, W, b):
    out, _ = _run(x, condition_ids, W, b)
    return out



# revision 3
# speedup vs baseline: 1.0451x; 1.0451x over previous
"""Conditional (per-row expert) linear layer for Trainium2, 8 NeuronCores.

Math: out[i] = W[c_i] @ x[i] + sum_c b[c]    (x: [B,D], W: [C,D,D], b: [C,D])

Strategy: expert-parallel (core c owns the rows with condition_ids == c,
host-gathered and padded to a common capacity n_cap so the SPMD NEFF has
static shapes), with the GEMM in TRANSPOSED orientation: the stationary
operand is a 128x128 W block [in-features x out-features] and the moving
operand is x with the batch in the FREE dimension. PE work is therefore
exactly (D/128)^2 * n_cap = 64*n_cap cycles/core -- no 128-row batch
quantization (the old row-tile layout paid ceil(n_cap/128) full tiles).

All device tensors are bf16 (1 cycle/row on the PE, same as float32r at
>=256 free, but half the HBM traffic: ~6.4 MB/core/exec vs 12.7). The
per-condition bias sum is added on the host in fp32 (free; out is linear).

Loop order per core: for m (8 out-feature tiles): for k (8 in-feature
tiles): matmul over all batch chunks (~260 cols each, PSUM-bank sized) --
so each stationary block is loaded once per (m,k) and streams n_cap rows,
keeping LDWEIGHTS fully hidden. PSUM accumulates over k; drains are pure
f32->bf16 copies alternating scalar/vector engines; W rides one HWDGE ring
(sync), x the SWDGE queues, outputs the scalar engine's HWDGE ring. W/x
SBUF tiles are double-buffered so back-to-back executions keep the PE
gapless (warm floor ~27.7us at 2.4 GHz for n_cap=1040).
"""

import sys
from contextlib import ExitStack

import numpy as np

try:
    import concourse.bass as bass  # noqa: F401
except ImportError:  # pragma: no cover
    sys.path.insert(0, "/opt/trn_rl_repo")

import jax
from jax.experimental.shard_map import shard_map
from jax.sharding import Mesh, PartitionSpec

import concourse.mybir as mybir
import concourse.tile as tile
from concourse import bacc
from concourse import bass2jax as _b2j

import ml_dtypes

B, D, C = 8192, 1024, 8
P = 128  # partitions
KT = D // P  # contraction tiles (input features / 128)
MT = D // P  # output-feature tiles
BF16 = np.dtype(ml_dtypes.bfloat16)

_cache: dict[tuple, "_Runner"] = {}


def _chunk_sizes(n: int) -> list[tuple[int, int]]:
    """Split n batch columns into (start, size) chunks, each <=512 (one PSUM
    bank of fp32) and a multiple of 4; ~260 nominal so LDWEIGHTS (128 rows)
    hides under every matmul and out-DMA lines stay >=512B."""
    assert n % 4 == 0
    nch = max(1, -(-n // 272))
    base = -(-(n // nch) // 4) * 4
    sizes = []
    left = n
    while left > 0:
        s = min(base, left, 512)
        sizes.append(s)
        left -= s
    out, pos = [], 0
    for s in sizes:
        out.append((pos, s))
        pos += s
    return out


def _build(n_cap: int, reps: int = 1):
    """Per-core program: Oimg[p, m*n_cap + i] = sum_kp W[m*128+p, kp] * x[i, kp].

    reps > 1 repeats the whole body (including all DMAs) back-to-back for
    benchmarking: wall(T) - wall(1) isolates per-execution device time."""
    chunks = _chunk_sizes(n_cap)
    nc = bacc.Bacc("TRN2", target_bir_lowering=False, debug=False, num_devices=8, num_swdge_queues=4)
    Xd = nc.dram_tensor("Ximg", [P, KT * n_cap], mybir.dt.bfloat16, kind="ExternalInput").ap()
    Wd = nc.dram_tensor("Wimg", [P, MT * D], mybir.dt.bfloat16, kind="ExternalInput").ap()
    Od = nc.dram_tensor("Oimg", [P, MT * n_cap], mybir.dt.bfloat16, kind="ExternalOutput").ap()

    with tile.TileContext(nc) as tc, ExitStack() as ctx:
        w_pool = ctx.enter_context(tc.tile_pool(name="w", bufs=2))
        x_pool = ctx.enter_context(tc.tile_pool(name="x", bufs=2))
        o_pool = ctx.enter_context(tc.tile_pool(name="o", bufs=2))
        ps_pool = ctx.enter_context(tc.tile_pool(name="ps", bufs=2, space="PSUM"))

        for _rep in range(reps):
            W_sb = w_pool.tile([P, MT * D], mybir.dt.bfloat16, name="W_sb", tag="wsb")
            # m0 lands in two pieces so the PE can start before the rest of
            # W; remaining m blocks are one 2KB-line transfer each.
            w_ranges = [(0, D // 2), (D // 2, D)] + [(m * D, (m + 1) * D) for m in range(1, MT)]
            for a, bnd in w_ranges:
                nc.sync.dma_start(W_sb[:, a:bnd], Wd[:, a:bnd])

            X_sb = x_pool.tile([P, KT * n_cap], mybir.dt.bfloat16, name="X_sb", tag="xsb")
            for s, cs in chunks:
                nc.gpsimd.dma_start(X_sb[:, KT * s : KT * (s + cs)], Xd[:, KT * s : KT * (s + cs)])

            for m in range(MT):
                ps = {}
                for ci in range(len(chunks)):
                    ps[ci] = ps_pool.tile([P, 512], mybir.dt.float32, name=f"ps{ci}", tag=f"ps{ci}")
                for k in range(KT):
                    wap = W_sb[:, m * D + k * P : m * D + (k + 1) * P]
                    for ci, (s, cs) in enumerate(chunks):
                        nc.tensor.matmul(
                            ps[ci][:, :cs],
                            wap,
                            X_sb[:, KT * s + k * cs : KT * s + (k + 1) * cs],
                            start=(k == 0),
                            stop=(k == KT - 1),
                            skip_group_check=True,
                        )
                for ci, (s, cs) in enumerate(chunks):
                    o_sb = o_pool.tile([P, 512], mybir.dt.bfloat16, name=f"o{ci}", tag=f"o{ci}")
                    if (m + ci) % 2 == 0:
                        nc.scalar.copy(o_sb[:, :cs], ps[ci][:, :cs])
                    else:
                        nc.vector.tensor_scalar_add(o_sb[:, :cs], ps[ci][:, :cs], 0.0)
                    nc.scalar.dma_start(Od[:, m * n_cap + s : m * n_cap + s + cs], o_sb[:, :cs])

    nc.compile()
    _check_noload_pairs(nc)
    return nc


def _check_noload_pairs(nc):
    """Every non-self-loading matmul must see the stationary operand its
    AP names actually loaded in the PE array -- for bf16, legalization
    emits a standalone InstLdweights before each InstMatmult, so track the
    last LDW (or self-loading matmul) and assert each matmul's stationary
    matches it. Scheduling is deterministic at build time, so passing here
    guarantees correctness on device."""
    loaded = None
    for fn in nc.m.functions:
        for blk in fn.blocks:
            for inst in blk.instructions:
                tn = type(inst).__name__
                if tn == "InstLdweights":
                    loaded = str(inst.ins[0])
                elif tn == "InstMatmult":
                    if inst.ldweights:
                        loaded = str(inst.ins[1])
                    else:
                        assert loaded is not None, "no-load matmul with nothing loaded"
                        assert str(inst.ins[1]) == loaded, (
                            f"no-load matmul stationary mismatch:\n"
                            f"loaded: {loaded}\nthis: {inst.ins[1]}"
                        )


class _Runner:
    """Caches the compiled NEFF + jitted shard_map executable for one n_cap."""

    def __init__(self, n_cap: int, reps: int = 1):
        self.n_cap = n_cap
        self.nc = _build(n_cap, reps)
        _b2j.install_neuronx_cc_hook()

        assert self.nc.dbg_addr is None
        partition_name = (
            self.nc.partition_id_tensor.name if self.nc.partition_id_tensor else None
        )

        in_names, out_names, out_avals = [], [], []
        for alloc in self.nc.m.functions[0].allocations:
            if not isinstance(alloc, mybir.MemoryLocationSet):
                continue
            name = alloc.memorylocations[0].name
            if alloc.kind == "ExternalInput":
                if name != partition_name:
                    in_names.append(name)
            elif alloc.kind == "ExternalOutput":
                out_names.append(name)
                out_avals.append(
                    jax.core.ShapedArray(
                        tuple(alloc.tensor_shape), mybir.dt.np(alloc.dtype)
                    )
                )
        self.in_names = in_names
        self.out_names = out_names
        self.out_avals = out_avals
        self.n_params = len(in_names)
        self.n_outs = len(out_names)
        all_in_names = tuple(in_names + out_names)
        if partition_name is not None:
            all_in_names = all_in_names + (partition_name,)

        nc = self.nc

        def _bind(*args):
            operands = list(args)
            if partition_name is not None:
                operands.append(_b2j.partition_id_tensor())
            return tuple(
                _b2j._bass_exec_p.bind(
                    *operands,
                    out_avals=tuple(out_avals),
                    in_names=all_in_names,
                    out_names=tuple(out_names),
                    lowering_input_output_aliases=(),
                    sim_require_finite=True,
                    sim_require_nnan=True,
                    nc=nc,
                )
            )

        self._bind = _bind
        self.devices = jax.devices("neuron")[:C]
        self.mesh = Mesh(np.asarray(self.devices), ("core",))
        spec_in = (PartitionSpec("core"),) * (self.n_params + self.n_outs)
        spec_out = (PartitionSpec("core"),) * self.n_outs
        self._spec_in, self._spec_out = spec_in, spec_out
        self._exec = jax.jit(
            shard_map(
                _bind,
                mesh=self.mesh,
                in_specs=spec_in,
                out_specs=spec_out,
                check_rep=False,
            ),
            donate_argnums=tuple(range(self.n_params, self.n_params + self.n_outs)),
            keep_unused=True,
        )

    def make_exec_nodonate(self):
        """Jitted executable that does not donate its output-init operands,
        so pre-staged device args can be reused across timing reps."""
        return jax.jit(
            shard_map(
                self._bind,
                mesh=self.mesh,
                in_specs=self._spec_in,
                out_specs=self._spec_out,
                check_rep=False,
            ),
            keep_unused=True,
        )

    def concat_inputs(self, in_maps):
        return [
            np.concatenate([np.asarray(m[name]) for m in in_maps], axis=0)
            for name in self.in_names
        ]

    def zero_outs(self):
        return [
            np.zeros((C * a.shape[0], *a.shape[1:]), a.dtype) for a in self.out_avals
        ]

    def run(self, in_maps):
        out_arrs = self._exec(*self.concat_inputs(in_maps), *self.zero_outs())
        return [
            {
                name: np.asarray(out_arrs[i]).reshape(C, *self.out_avals[i].shape)[c]
                for i, name in enumerate(self.out_names)
            }
            for c in range(C)
        ]


def _get(n_cap: int, reps: int = 1) -> _Runner:
    key = (n_cap, reps)
    if key not in _cache:
        _cache[key] = _Runner(n_cap, reps)
    return _cache[key]


def _prep(x, condition_ids, W, b):
    x = np.ascontiguousarray(np.asarray(x, dtype=np.float32))
    cond = np.asarray(condition_ids).astype(np.int64)
    W = np.asarray(W, dtype=np.float32)
    b = np.asarray(b, dtype=np.float32)

    bias_sum = b.sum(axis=0, dtype=np.float64).astype(np.float32)  # [D]

    rows = [np.nonzero(cond == c)[0] for c in range(C)]
    n_max = max(len(r) for r in rows)
    n_cap = max(16, -(-n_max // 16) * 16)
    chunks = _chunk_sizes(n_cap)

    in_maps = []
    for c in range(C):
        r = rows[c]
        xg = np.zeros((n_cap, D), np.float32)
        xg[: len(r)] = x[r]
        X3 = xg.astype(BF16).reshape(n_cap, KT, P)
        Ximg = np.empty((P, KT * n_cap), BF16)
        for s, cs in chunks:
            Ximg[:, KT * s : KT * (s + cs)] = (
                X3[s : s + cs].transpose(2, 1, 0).reshape(P, KT * cs)
            )
        # Wimg[p, m*D + k*P + j] = W[c][m*P + j, k*P + p]
        Wimg = np.ascontiguousarray(
            W[c].astype(BF16).reshape(MT, P, KT, P).transpose(3, 0, 2, 1).reshape(P, MT * D)
        )
        in_maps.append({"Ximg": Ximg, "Wimg": Wimg})
    return rows, n_cap, in_maps, bias_sum


def _postprocess(out_imgs, rows, n_cap, bias_sum):
    """out_imgs: per-core [P, MT*n_cap] bf16 -> full [B, D] fp32 output."""
    out = np.empty((B, D), np.float32)
    for c in range(C):
        r = rows[c]
        if len(r) == 0:
            continue
        O = np.asarray(out_imgs[c]).reshape(P, MT, n_cap)[:, :, : len(r)]
        res = O.transpose(2, 1, 0).reshape(len(r), D).astype(np.float32)
        out[r] = res + bias_sum
    return out


def _run(x, condition_ids, W, b):
    rows, n_cap, in_maps, bias_sum = _prep(x, condition_ids, W, b)
    runner = _get(n_cap)
    results = runner.run(in_maps)
    out = _postprocess([results[c]["Oimg"] for c in range(C)], rows, n_cap, bias_sum)
    return out, runner


def kernel(x, condition_ids, W, b):
    out, _ = _run(x, condition_ids, W, b)
    return out


# revision 5
# speedup vs baseline: 1.1197x; 1.0713x over previous
"""Conditional (per-row expert) linear layer for Trainium2, 8 NeuronCores.

Math: out[i] = W[c_i] @ x[i] + sum_c b[c]    (x: [B,D], W: [C,D,D], b: [C,D])

Strategy: expert-parallel (core c owns the rows with condition_ids == c,
host-gathered and padded to a common capacity n_cap so the SPMD NEFF has
static shapes), with the GEMM in TRANSPOSED orientation: the stationary
operand is a 128x128 W block [in-features x out-features] and the moving
operand is x with the batch in the FREE dimension. PE work is therefore
exactly (D/128)^2 * n_cap = 64*n_cap cycles/core -- no 128-row batch
quantization (the old row-tile layout paid ceil(n_cap/128) full tiles).

All device tensors are bf16 (1 cycle/row on the PE, same as float32r at
>=256 free, but half the HBM traffic: ~6.4 MB/core/exec vs 12.7). The
per-condition bias sum is added on the host in fp32 (free; out is linear).

Loop order per core: for m (8 out-feature tiles): for k (8 in-feature
tiles): matmul over all batch chunks (~260 cols each, PSUM-bank sized) --
so each stationary block is loaded once per (m,k) and streams n_cap rows,
keeping LDWEIGHTS fully hidden. PSUM accumulates over k; drains are pure
f32->bf16 copies alternating scalar/vector engines; W rides one HWDGE ring
(sync), x the SWDGE queues, outputs the scalar engine's HWDGE ring. W/x
SBUF tiles are double-buffered so back-to-back executions keep the PE
gapless (warm floor ~27.7us at 2.4 GHz for n_cap=1040).
"""

import sys
from contextlib import ExitStack

import numpy as np

try:
    import concourse.bass as bass  # noqa: F401
except ImportError:  # pragma: no cover
    sys.path.insert(0, "/opt/trn_rl_repo")

import jax
from jax.experimental.shard_map import shard_map
from jax.sharding import Mesh, PartitionSpec

import concourse.mybir as mybir
import concourse.tile as tile
from concourse import bacc
from concourse import bass2jax as _b2j

import ml_dtypes

B, D, C = 8192, 1024, 8
P = 128  # partitions
KT = D // P  # contraction tiles (input features / 128)
MT = D // P  # output-feature tiles
BF16 = np.dtype(ml_dtypes.bfloat16)

_cache: dict[tuple, "_Runner"] = {}


def _chunk_sizes(n: int) -> list[tuple[int, int]]:
    """Split n batch columns into (start, size) chunks, each <=512 (one PSUM
    bank of fp32) and a multiple of 4; ~260 nominal so LDWEIGHTS (128 rows)
    hides under every matmul and out-DMA lines stay >=512B."""
    assert n % 4 == 0
    nch = max(1, -(-n // 272))
    base = -(-(n // nch) // 4) * 4
    sizes = []
    left = n
    while left > 0:
        s = min(base, left, 512)
        sizes.append(s)
        left -= s
    out, pos = [], 0
    for s in sizes:
        out.append((pos, s))
        pos += s
    return out


def _build(n_cap: int, reps: int = 1):
    """Per-core program: Oimg[p, m*n_cap + i] = sum_kp W[m*128+p, kp] * x[i, kp].

    reps > 1 repeats the whole body (including all DMAs) back-to-back for
    benchmarking: wall(T) - wall(1) isolates per-execution device time."""
    chunks = _chunk_sizes(n_cap)
    nc = bacc.Bacc("TRN2", target_bir_lowering=False, debug=False, num_devices=8, num_swdge_queues=4)
    Xd = nc.dram_tensor("Ximg", [P, KT * n_cap], mybir.dt.bfloat16, kind="ExternalInput").ap()
    Wd = nc.dram_tensor("Wimg", [P, MT * D], mybir.dt.bfloat16, kind="ExternalInput").ap()
    Od = nc.dram_tensor("Oimg", [P, MT * n_cap], mybir.dt.bfloat16, kind="ExternalOutput").ap()

    with tile.TileContext(nc) as tc, ExitStack() as ctx:
        w_pool = ctx.enter_context(tc.tile_pool(name="w", bufs=2))
        x_pool = ctx.enter_context(tc.tile_pool(name="x", bufs=2))
        o_pool = ctx.enter_context(tc.tile_pool(name="o", bufs=2))
        ps_pool = ctx.enter_context(tc.tile_pool(name="ps", bufs=2, space="PSUM"))

        for _rep in range(reps):
            W_sb = w_pool.tile([P, MT * D], mybir.dt.bfloat16, name="W_sb", tag="wsb")
            # m0 lands in two pieces so the PE can start before the rest of
            # W; remaining m blocks are one 2KB-line transfer each.
            w_ranges = [(0, D // 2), (D // 2, D)] + [(m * D, (m + 1) * D) for m in range(1, MT)]
            for a, bnd in w_ranges:
                nc.sync.dma_start(W_sb[:, a:bnd], Wd[:, a:bnd])

            X_sb = x_pool.tile([P, KT * n_cap], mybir.dt.bfloat16, name="X_sb", tag="xsb")
            for s, cs in chunks:
                nc.gpsimd.dma_start(X_sb[:, KT * s : KT * (s + cs)], Xd[:, KT * s : KT * (s + cs)])

            for m in range(MT):
                ps = {}
                for ci in range(len(chunks)):
                    ps[ci] = ps_pool.tile([P, 512], mybir.dt.float32, name=f"ps{ci}", tag=f"ps{ci}")
                for k in range(KT):
                    wap = W_sb[:, m * D + k * P : m * D + (k + 1) * P]
                    for ci, (s, cs) in enumerate(chunks):
                        nc.tensor.matmul(
                            ps[ci][:, :cs],
                            wap,
                            X_sb[:, KT * s + k * cs : KT * s + (k + 1) * cs],
                            start=(k == 0),
                            stop=(k == KT - 1),
                            skip_group_check=True,
                        )
                for ci, (s, cs) in enumerate(chunks):
                    o_sb = o_pool.tile([P, 512], mybir.dt.bfloat16, name=f"o{ci}", tag=f"o{ci}")
                    if (m + ci) % 2 == 0:
                        nc.scalar.copy(o_sb[:, :cs], ps[ci][:, :cs])
                    else:
                        nc.vector.tensor_scalar_add(o_sb[:, :cs], ps[ci][:, :cs], 0.0)
                    nc.scalar.dma_start(Od[:, m * n_cap + s : m * n_cap + s + cs], o_sb[:, :cs])

    nc.compile()
    _dedup_ldweights(nc)
    _check_noload_pairs(nc)
    return nc


def _dedup_ldweights(nc):
    """tile_legalize emits one InstLdweights per bf16 matmul with no
    redundancy elision; back-to-back reloads of the already-loaded
    stationary AP cost ~53ns each (FWL) of pure PE stall. Delete every
    Ldweights whose AP is already loaded and that carries no semaphore
    wait/update (scheduling metadata only -- runtime order is unchanged)."""
    removed = 0
    for fn in nc.m.functions:
        for blk in fn.blocks:
            loaded = None
            keep = []
            n_before = len(blk.instructions)
            for inst in blk.instructions:
                tn = type(inst).__name__
                if tn == "InstLdweights":
                    key = str(inst.ins[0])
                    if key == loaded and not inst.has_wait() and not inst.has_update():
                        continue
                    loaded = key
                elif tn == "InstMatmult":
                    if inst.ldweights:
                        loaded = str(inst.ins[1])
                elif tn == "InstMatmultMx":
                    loaded = None
                keep.append(inst)
            if len(keep) != n_before:
                blk.instructions = keep
                removed += n_before - len(keep)
    return removed


def _check_noload_pairs(nc):
    """Every non-self-loading matmul must see the stationary operand its
    AP names actually loaded in the PE array -- for bf16, legalization
    emits a standalone InstLdweights before each InstMatmult, so track the
    last LDW (or self-loading matmul) and assert each matmul's stationary
    matches it. Scheduling is deterministic at build time, so passing here
    guarantees correctness on device."""
    loaded = None
    for fn in nc.m.functions:
        for blk in fn.blocks:
            for inst in blk.instructions:
                tn = type(inst).__name__
                if tn == "InstLdweights":
                    loaded = str(inst.ins[0])
                elif tn == "InstMatmult":
                    if inst.ldweights:
                        loaded = str(inst.ins[1])
                    else:
                        assert loaded is not None, "no-load matmul with nothing loaded"
                        assert str(inst.ins[1]) == loaded, (
                            f"no-load matmul stationary mismatch:\n"
                            f"loaded: {loaded}\nthis: {inst.ins[1]}"
                        )


class _Runner:
    """Caches the compiled NEFF + jitted shard_map executable for one n_cap."""

    def __init__(self, n_cap: int, reps: int = 1):
        self.n_cap = n_cap
        self.nc = _build(n_cap, reps)
        _b2j.install_neuronx_cc_hook()

        assert self.nc.dbg_addr is None
        partition_name = (
            self.nc.partition_id_tensor.name if self.nc.partition_id_tensor else None
        )

        in_names, out_names, out_avals = [], [], []
        for alloc in self.nc.m.functions[0].allocations:
            if not isinstance(alloc, mybir.MemoryLocationSet):
                continue
            name = alloc.memorylocations[0].name
            if alloc.kind == "ExternalInput":
                if name != partition_name:
                    in_names.append(name)
            elif alloc.kind == "ExternalOutput":
                out_names.append(name)
                out_avals.append(
                    jax.core.ShapedArray(
                        tuple(alloc.tensor_shape), mybir.dt.np(alloc.dtype)
                    )
                )
        self.in_names = in_names
        self.out_names = out_names
        self.out_avals = out_avals
        self.n_params = len(in_names)
        self.n_outs = len(out_names)
        all_in_names = tuple(in_names + out_names)
        if partition_name is not None:
            all_in_names = all_in_names + (partition_name,)

        nc = self.nc

        def _bind(*args):
            operands = list(args)
            if partition_name is not None:
                operands.append(_b2j.partition_id_tensor())
            return tuple(
                _b2j._bass_exec_p.bind(
                    *operands,
                    out_avals=tuple(out_avals),
                    in_names=all_in_names,
                    out_names=tuple(out_names),
                    lowering_input_output_aliases=(),
                    sim_require_finite=True,
                    sim_require_nnan=True,
                    nc=nc,
                )
            )

        self._bind = _bind
        self.devices = jax.devices("neuron")[:C]
        self.mesh = Mesh(np.asarray(self.devices), ("core",))
        spec_in = (PartitionSpec("core"),) * (self.n_params + self.n_outs)
        spec_out = (PartitionSpec("core"),) * self.n_outs
        self._spec_in, self._spec_out = spec_in, spec_out
        self._exec = jax.jit(
            shard_map(
                _bind,
                mesh=self.mesh,
                in_specs=spec_in,
                out_specs=spec_out,
                check_rep=False,
            ),
            donate_argnums=tuple(range(self.n_params, self.n_params + self.n_outs)),
            keep_unused=True,
        )

    def make_exec_nodonate(self):
        """Jitted executable that does not donate its output-init operands,
        so pre-staged device args can be reused across timing reps."""
        return jax.jit(
            shard_map(
                self._bind,
                mesh=self.mesh,
                in_specs=self._spec_in,
                out_specs=self._spec_out,
                check_rep=False,
            ),
            keep_unused=True,
        )

    def concat_inputs(self, in_maps):
        return [
            np.concatenate([np.asarray(m[name]) for m in in_maps], axis=0)
            for name in self.in_names
        ]

    def zero_outs(self):
        return [
            np.zeros((C * a.shape[0], *a.shape[1:]), a.dtype) for a in self.out_avals
        ]

    def run(self, in_maps):
        out_arrs = self._exec(*self.concat_inputs(in_maps), *self.zero_outs())
        return [
            {
                name: np.asarray(out_arrs[i]).reshape(C, *self.out_avals[i].shape)[c]
                for i, name in enumerate(self.out_names)
            }
            for c in range(C)
        ]


def _get(n_cap: int, reps: int = 1) -> _Runner:
    key = (n_cap, reps)
    if key not in _cache:
        _cache[key] = _Runner(n_cap, reps)
    return _cache[key]


def _prep(x, condition_ids, W, b):
    x = np.ascontiguousarray(np.asarray(x, dtype=np.float32))
    cond = np.asarray(condition_ids).astype(np.int64)
    W = np.asarray(W, dtype=np.float32)
    b = np.asarray(b, dtype=np.float32)

    bias_sum = b.sum(axis=0, dtype=np.float64).astype(np.float32)  # [D]

    rows = [np.nonzero(cond == c)[0] for c in range(C)]
    n_max = max(len(r) for r in rows)
    n_cap = max(16, -(-n_max // 16) * 16)
    chunks = _chunk_sizes(n_cap)

    in_maps = []
    for c in range(C):
        r = rows[c]
        xg = np.zeros((n_cap, D), np.float32)
        xg[: len(r)] = x[r]
        X3 = xg.astype(BF16).reshape(n_cap, KT, P)
        Ximg = np.empty((P, KT * n_cap), BF16)
        for s, cs in chunks:
            Ximg[:, KT * s : KT * (s + cs)] = (
                X3[s : s + cs].transpose(2, 1, 0).reshape(P, KT * cs)
            )
        # Wimg[p, m*D + k*P + j] = W[c][m*P + j, k*P + p]
        Wimg = np.ascontiguousarray(
            W[c].astype(BF16).reshape(MT, P, KT, P).transpose(3, 0, 2, 1).reshape(P, MT * D)
        )
        in_maps.append({"Ximg": Ximg, "Wimg": Wimg})
    return rows, n_cap, in_maps, bias_sum


def _postprocess(out_imgs, rows, n_cap, bias_sum):
    """out_imgs: per-core [P, MT*n_cap] bf16 -> full [B, D] fp32 output."""
    out = np.empty((B, D), np.float32)
    for c in range(C):
        r = rows[c]
        if len(r) == 0:
            continue
        O = np.asarray(out_imgs[c]).reshape(P, MT, n_cap)[:, :, : len(r)]
        res = O.transpose(2, 1, 0).reshape(len(r), D).astype(np.float32)
        out[r] = res + bias_sum
    return out


def _run(x, condition_ids, W, b):
    rows, n_cap, in_maps, bias_sum = _prep(x, condition_ids, W, b)
    runner = _get(n_cap)
    results = runner.run(in_maps)
    out = _postprocess([results[c]["Oimg"] for c in range(C)], rows, n_cap, bias_sum)
    return out, runner


def kernel(x, condition_ids, W, b):
    out, _ = _run(x, condition_ids, W, b)
    return out


# revision 8
# speedup vs baseline: 1.1649x; 1.0404x over previous
"""Conditional (per-row expert) linear layer for Trainium2, 8 NeuronCores.

Math: out[i] = W[c_i] @ x[i] + sum_c b[c]    (x: [B,D], W: [C,D,D], b: [C,D])

Strategy: expert-parallel (core c owns the rows with condition_ids == c,
host-gathered and padded to a common capacity n_cap so the SPMD NEFF has
static shapes), with the GEMM in TRANSPOSED orientation: the stationary
operand is a 128x128 W block [in-features x out-features] and the moving
operand is x with the batch in the FREE dimension. PE work is therefore
exactly (D/128)^2 * n_cap = 64*n_cap cycles/core -- no 128-row batch
quantization (the old row-tile layout paid ceil(n_cap/128) full tiles).

All device tensors are bf16 (1 cycle/row on the PE, same as float32r at
>=256 free, but half the HBM traffic: ~6.4 MB/core/exec vs 12.7). The
per-condition bias sum is added on the host in fp32 (free; out is linear).

Loop order per core: for m (8 out-feature tiles): for k (8 in-feature
tiles): matmul over all batch chunks (~260 cols each, PSUM-bank sized) --
so each stationary block is loaded once per (m,k) and streams n_cap rows,
keeping LDWEIGHTS fully hidden. PSUM accumulates over k; drains are pure
f32->bf16 copies alternating scalar/vector engines; W rides one HWDGE ring
(sync), x the SWDGE queues, outputs the scalar engine's HWDGE ring. W/x
SBUF tiles are double-buffered so back-to-back executions keep the PE
gapless (warm floor ~27.7us at 2.4 GHz for n_cap=1040).
"""

import sys
from contextlib import ExitStack

import numpy as np

try:
    import concourse.bass as bass  # noqa: F401
except ImportError:  # pragma: no cover
    sys.path.insert(0, "/opt/trn_rl_repo")

import jax
from jax.experimental.shard_map import shard_map
from jax.sharding import Mesh, PartitionSpec

import concourse.mybir as mybir
import concourse.tile as tile
from concourse import bacc
from concourse import bass2jax as _b2j

import ml_dtypes

B, D, C = 8192, 1024, 8
P = 128  # partitions
KT = D // P  # contraction tiles (input features / 128)
MT = D // P  # output-feature tiles
BF16 = np.dtype(ml_dtypes.bfloat16)

_cache: dict[tuple, "_Runner"] = {}


def _chunk_sizes(n: int) -> list[tuple[int, int]]:
    """Split n batch columns into (start, size) chunks, each <=512 (one PSUM
    bank of fp32) and a multiple of 4; ~348 nominal (3 chunks at n~1040) so
    PSUM fits 2x-buffered chunk tiles and per-chunk work stays coarse."""
    assert n % 4 == 0
    nch = max(1, -(-n // 352))
    base = -(-(n // nch) // 4) * 4
    sizes = []
    left = n
    while left > 0:
        s = min(base, left, 512)
        sizes.append(s)
        left -= s
    out, pos = [], 0
    for s in sizes:
        out.append((pos, s))
        pos += s
    return out


def _build(n_cap: int, reps: int = 1):
    """Per-core program: Oimg[p, m*n_cap + i] = sum_kp W[m*128+p, kp] * x[i, kp].

    reps > 1 repeats the whole body (including all DMAs) back-to-back for
    benchmarking: wall(T) - wall(1) isolates per-execution device time."""
    chunks = _chunk_sizes(n_cap)
    nc = bacc.Bacc("TRN2", target_bir_lowering=False, debug=False, num_devices=8, num_swdge_queues=4)
    Xd = nc.dram_tensor("Ximg", [P, KT * n_cap], mybir.dt.bfloat16, kind="ExternalInput").ap()
    Wd = nc.dram_tensor("Wimg", [P, MT * D], mybir.dt.bfloat16, kind="ExternalInput").ap()
    Od = nc.dram_tensor("Oimg", [P, MT * n_cap], mybir.dt.bfloat16, kind="ExternalOutput").ap()

    with tile.TileContext(nc) as tc, ExitStack() as ctx:
        w_pool = ctx.enter_context(tc.tile_pool(name="w", bufs=2))
        x_pool = ctx.enter_context(tc.tile_pool(name="x", bufs=2))
        o_pool = ctx.enter_context(tc.tile_pool(name="o", bufs=2))
        ps_pool = ctx.enter_context(tc.tile_pool(name="ps", bufs=2, space="PSUM"))

        for _rep in range(reps):
            W_sb = w_pool.tile([P, MT * D], mybir.dt.bfloat16, name="W_sb", tag="wsb")
            # m0 lands in two pieces so the PE can start before the rest of
            # W; remaining m blocks are one 2KB-line transfer each.
            w_ranges = [(0, D // 2), (D // 2, D)] + [(m * D, (m + 1) * D) for m in range(1, MT)]
            for a, bnd in w_ranges:
                nc.sync.dma_start(W_sb[:, a:bnd], Wd[:, a:bnd])

            X_sb = x_pool.tile([P, KT * n_cap], mybir.dt.bfloat16, name="X_sb", tag="xsb")
            # two SWDGE pieces along chunk-region boundaries
            half = len(chunks) - len(chunks) // 2
            for grp in (chunks[:half], chunks[half:]):
                if not grp:
                    continue
                a = KT * grp[0][0]
                bnd = KT * (grp[-1][0] + grp[-1][1])
                nc.gpsimd.dma_start(X_sb[:, a:bnd], Xd[:, a:bnd])

            for m in range(MT):
                ps = {}
                for ci in range(len(chunks)):
                    ps[ci] = ps_pool.tile([P, 512], mybir.dt.float32, name=f"ps{ci}", tag=f"ps{ci}")
                for k in range(KT):
                    wap = W_sb[:, m * D + k * P : m * D + (k + 1) * P]
                    for ci, (s, cs) in enumerate(chunks):
                        nc.tensor.matmul(
                            ps[ci][:, :cs],
                            wap,
                            X_sb[:, KT * s + k * cs : KT * s + (k + 1) * cs],
                            start=(k == 0),
                            stop=(k == KT - 1),
                            skip_group_check=True,
                        )
                o_sb = o_pool.tile([P, n_cap], mybir.dt.bfloat16, name="om", tag="om")
                for ci, (s, cs) in enumerate(chunks):
                    if (m + ci) % 2 == 0:
                        nc.scalar.copy(o_sb[:, s : s + cs], ps[ci][:, :cs])
                    else:
                        nc.vector.tensor_scalar_add(o_sb[:, s : s + cs], ps[ci][:, :cs], 0.0)
                nc.scalar.dma_start(Od[:, m * n_cap : (m + 1) * n_cap], o_sb[:])

    nc.compile()
    _dedup_ldweights(nc)
    _check_noload_pairs(nc)
    return nc


def _dedup_ldweights(nc):
    """tile_legalize emits one InstLdweights per bf16 matmul with no
    redundancy elision; back-to-back reloads of the already-loaded
    stationary AP cost ~53ns each (FWL) of pure PE stall. Delete every
    Ldweights whose AP is already loaded and that carries no semaphore
    wait/update (scheduling metadata only -- runtime order is unchanged)."""
    removed = 0
    for fn in nc.m.functions:
        for blk in fn.blocks:
            loaded = None
            keep = []
            n_before = len(blk.instructions)
            for inst in blk.instructions:
                tn = type(inst).__name__
                if tn == "InstLdweights":
                    key = str(inst.ins[0])
                    if key == loaded and not inst.has_wait() and not inst.has_update():
                        continue
                    loaded = key
                elif tn == "InstMatmult":
                    if inst.ldweights:
                        loaded = str(inst.ins[1])
                elif tn == "InstMatmultMx":
                    loaded = None
                keep.append(inst)
            if len(keep) != n_before:
                blk.instructions = keep
                removed += n_before - len(keep)
    return removed


def _check_noload_pairs(nc):
    """Every non-self-loading matmul must see the stationary operand its
    AP names actually loaded in the PE array -- for bf16, legalization
    emits a standalone InstLdweights before each InstMatmult, so track the
    last LDW (or self-loading matmul) and assert each matmul's stationary
    matches it. Scheduling is deterministic at build time, so passing here
    guarantees correctness on device."""
    loaded = None
    for fn in nc.m.functions:
        for blk in fn.blocks:
            for inst in blk.instructions:
                tn = type(inst).__name__
                if tn == "InstLdweights":
                    loaded = str(inst.ins[0])
                elif tn == "InstMatmult":
                    if inst.ldweights:
                        loaded = str(inst.ins[1])
                    else:
                        assert loaded is not None, "no-load matmul with nothing loaded"
                        assert str(inst.ins[1]) == loaded, (
                            f"no-load matmul stationary mismatch:\n"
                            f"loaded: {loaded}\nthis: {inst.ins[1]}"
                        )


class _Runner:
    """Caches the compiled NEFF + jitted shard_map executable for one n_cap."""

    def __init__(self, n_cap: int, reps: int = 1):
        self.n_cap = n_cap
        self.nc = _build(n_cap, reps)
        _b2j.install_neuronx_cc_hook()

        assert self.nc.dbg_addr is None
        partition_name = (
            self.nc.partition_id_tensor.name if self.nc.partition_id_tensor else None
        )

        in_names, out_names, out_avals = [], [], []
        for alloc in self.nc.m.functions[0].allocations:
            if not isinstance(alloc, mybir.MemoryLocationSet):
                continue
            name = alloc.memorylocations[0].name
            if alloc.kind == "ExternalInput":
                if name != partition_name:
                    in_names.append(name)
            elif alloc.kind == "ExternalOutput":
                out_names.append(name)
                out_avals.append(
                    jax.core.ShapedArray(
                        tuple(alloc.tensor_shape), mybir.dt.np(alloc.dtype)
                    )
                )
        self.in_names = in_names
        self.out_names = out_names
        self.out_avals = out_avals
        self.n_params = len(in_names)
        self.n_outs = len(out_names)
        all_in_names = tuple(in_names + out_names)
        if partition_name is not None:
            all_in_names = all_in_names + (partition_name,)

        nc = self.nc

        def _bind(*args):
            operands = list(args)
            if partition_name is not None:
                operands.append(_b2j.partition_id_tensor())
            return tuple(
                _b2j._bass_exec_p.bind(
                    *operands,
                    out_avals=tuple(out_avals),
                    in_names=all_in_names,
                    out_names=tuple(out_names),
                    lowering_input_output_aliases=(),
                    sim_require_finite=True,
                    sim_require_nnan=True,
                    nc=nc,
                )
            )

        self._bind = _bind
        self.devices = jax.devices("neuron")[:C]
        self.mesh = Mesh(np.asarray(self.devices), ("core",))
        spec_in = (PartitionSpec("core"),) * (self.n_params + self.n_outs)
        spec_out = (PartitionSpec("core"),) * self.n_outs
        self._spec_in, self._spec_out = spec_in, spec_out
        self._exec = jax.jit(
            shard_map(
                _bind,
                mesh=self.mesh,
                in_specs=spec_in,
                out_specs=spec_out,
                check_rep=False,
            ),
            donate_argnums=tuple(range(self.n_params, self.n_params + self.n_outs)),
            keep_unused=True,
        )

    def make_exec_nodonate(self):
        """Jitted executable that does not donate its output-init operands,
        so pre-staged device args can be reused across timing reps."""
        return jax.jit(
            shard_map(
                self._bind,
                mesh=self.mesh,
                in_specs=self._spec_in,
                out_specs=self._spec_out,
                check_rep=False,
            ),
            keep_unused=True,
        )

    def concat_inputs(self, in_maps):
        return [
            np.concatenate([np.asarray(m[name]) for m in in_maps], axis=0)
            for name in self.in_names
        ]

    def zero_outs(self):
        return [
            np.zeros((C * a.shape[0], *a.shape[1:]), a.dtype) for a in self.out_avals
        ]

    def run(self, in_maps):
        out_arrs = self._exec(*self.concat_inputs(in_maps), *self.zero_outs())
        return [
            {
                name: np.asarray(out_arrs[i]).reshape(C, *self.out_avals[i].shape)[c]
                for i, name in enumerate(self.out_names)
            }
            for c in range(C)
        ]


def _get(n_cap: int, reps: int = 1) -> _Runner:
    key = (n_cap, reps)
    if key not in _cache:
        _cache[key] = _Runner(n_cap, reps)
    return _cache[key]


def _prep(x, condition_ids, W, b):
    x = np.ascontiguousarray(np.asarray(x, dtype=np.float32))
    cond = np.asarray(condition_ids).astype(np.int64)
    W = np.asarray(W, dtype=np.float32)
    b = np.asarray(b, dtype=np.float32)

    bias_sum = b.sum(axis=0, dtype=np.float64).astype(np.float32)  # [D]

    rows = [np.nonzero(cond == c)[0] for c in range(C)]
    n_max = max(len(r) for r in rows)
    n_cap = max(16, -(-n_max // 16) * 16)
    chunks = _chunk_sizes(n_cap)

    in_maps = []
    for c in range(C):
        r = rows[c]
        xg = np.zeros((n_cap, D), np.float32)
        xg[: len(r)] = x[r]
        X3 = xg.astype(BF16).reshape(n_cap, KT, P)
        Ximg = np.empty((P, KT * n_cap), BF16)
        for s, cs in chunks:
            Ximg[:, KT * s : KT * (s + cs)] = (
                X3[s : s + cs].transpose(2, 1, 0).reshape(P, KT * cs)
            )
        # Wimg[p, m*D + k*P + j] = W[c][m*P + j, k*P + p]
        Wimg = np.ascontiguousarray(
            W[c].astype(BF16).reshape(MT, P, KT, P).transpose(3, 0, 2, 1).reshape(P, MT * D)
        )
        in_maps.append({"Ximg": Ximg, "Wimg": Wimg})
    return rows, n_cap, in_maps, bias_sum


def _postprocess(out_imgs, rows, n_cap, bias_sum):
    """out_imgs: per-core [P, MT*n_cap] bf16 -> full [B, D] fp32 output."""
    out = np.empty((B, D), np.float32)
    for c in range(C):
        r = rows[c]
        if len(r) == 0:
            continue
        O = np.asarray(out_imgs[c]).reshape(P, MT, n_cap)[:, :, : len(r)]
        res = O.transpose(2, 1, 0).reshape(len(r), D).astype(np.float32)
        out[r] = res + bias_sum
    return out


def _run(x, condition_ids, W, b):
    rows, n_cap, in_maps, bias_sum = _prep(x, condition_ids, W, b)
    runner = _get(n_cap)
    results = runner.run(in_maps)
    out = _postprocess([results[c]["Oimg"] for c in range(C)], rows, n_cap, bias_sum)
    return out, runner


def kernel(x, condition_ids, W, b):
    out, _ = _run(x, condition_ids, W, b)
    return out


# revision 10
# speedup vs baseline: 1.2027x; 1.0325x over previous
"""Conditional (per-row expert) linear layer for Trainium2, 8 NeuronCores.

Math: out[i] = W[c_i] @ x[i] + sum_c b[c]    (x: [B,D], W: [C,D,D], b: [C,D])

Strategy: expert-parallel (core c owns the rows with condition_ids == c,
host-gathered and padded to a common capacity n_cap so the SPMD NEFF has
static shapes), with the GEMM in TRANSPOSED orientation: the stationary
operand is a 128x128 W block [in-features x out-features] and the moving
operand is x with the batch in the FREE dimension. PE work is therefore
exactly (D/128)^2 * n_cap = 64*n_cap cycles/core -- no 128-row batch
quantization (the old row-tile layout paid ceil(n_cap/128) full tiles).

All device tensors are bf16 (1 cycle/row on the PE, same as float32r at
>=256 free, but half the HBM traffic: ~6.4 MB/core/exec vs 12.7). The
per-condition bias sum is added on the host in fp32 (free; out is linear).

Loop order per core: for m (8 out-feature tiles): for k (8 in-feature
tiles): matmul over all batch chunks (~260 cols each, PSUM-bank sized) --
so each stationary block is loaded once per (m,k) and streams n_cap rows,
keeping LDWEIGHTS fully hidden. PSUM accumulates over k; drains are pure
f32->bf16 copies alternating scalar/vector engines; W rides one HWDGE ring
(sync), x the SWDGE queues, outputs the scalar engine's HWDGE ring. W/x
SBUF tiles are double-buffered so back-to-back executions keep the PE
gapless (warm floor ~27.7us at 2.4 GHz for n_cap=1040).
"""

import sys
from contextlib import ExitStack

import numpy as np

try:
    import concourse.bass as bass  # noqa: F401
except ImportError:  # pragma: no cover
    sys.path.insert(0, "/opt/trn_rl_repo")

import jax
from jax.experimental.shard_map import shard_map
from jax.sharding import Mesh, PartitionSpec

import concourse.mybir as mybir
import concourse.tile as tile
from concourse import bacc
from concourse import bass2jax as _b2j

import ml_dtypes

B, D, C = 8192, 1024, 8
P = 128  # partitions
KT = D // P  # contraction tiles (input features / 128)
MT = D // P  # output-feature tiles
BF16 = np.dtype(ml_dtypes.bfloat16)

_cache: dict[tuple, "_Runner"] = {}
_FIXED_K_PROBE = False  # diagnostic only: reuse k=0 stationary (wrong math)


def _chunk_sizes(n: int) -> list[tuple[int, int]]:
    """Split n batch columns into (start, size) chunks, each <=512 (one PSUM
    bank of fp32) and a multiple of 4; ~348 nominal (3 chunks at n~1040) so
    PSUM fits 2x-buffered chunk tiles and per-chunk work stays coarse."""
    assert n % 4 == 0
    nch = max(1, -(-n // 352))
    base = -(-(n // nch) // 4) * 4
    sizes = []
    left = n
    while left > 0:
        s = min(base, left, 512)
        sizes.append(s)
        left -= s
    out, pos = [], 0
    for s in sizes:
        out.append((pos, s))
        pos += s
    return out


def _build(n_cap: int, reps: int = 1):
    """Per-core program: Oimg[p, m*n_cap + i] = sum_kp W[m*128+p, kp] * x[i, kp].

    reps > 1 repeats the whole body (including all DMAs) back-to-back for
    benchmarking: wall(T) - wall(1) isolates per-execution device time."""
    chunks = _chunk_sizes(n_cap)
    nc = bacc.Bacc("TRN2", target_bir_lowering=False, debug=False, num_devices=8, num_swdge_queues=4)
    Xd = nc.dram_tensor("Ximg", [P, KT * n_cap], mybir.dt.bfloat16, kind="ExternalInput").ap()
    Wd = nc.dram_tensor("Wimg", [P, MT * D], mybir.dt.bfloat16, kind="ExternalInput").ap()
    Od = nc.dram_tensor("Oimg", [P, MT * n_cap], mybir.dt.bfloat16, kind="ExternalOutput").ap()

    with tile.TileContext(nc) as tc, ExitStack() as ctx:
        w_pool = ctx.enter_context(tc.tile_pool(name="w", bufs=2))
        x_pool = ctx.enter_context(tc.tile_pool(name="x", bufs=2))
        o_pool = ctx.enter_context(tc.tile_pool(name="o", bufs=2))
        ps_pool = ctx.enter_context(tc.tile_pool(name="ps", bufs=2, space="PSUM"))

        for _rep in range(reps):
            W_sb = w_pool.tile([P, MT * D], mybir.dt.bfloat16, name="W_sb", tag="wsb")
            # m0 lands in two pieces so the PE can start before the rest of
            # W; remaining m blocks are one 2KB-line transfer each.
            w_ranges = [(0, D // 2), (D // 2, D)] + [(m * D, (m + 1) * D) for m in range(1, MT)]
            for a, bnd in w_ranges:
                nc.sync.dma_start(W_sb[:, a:bnd], Wd[:, a:bnd])

            X_sb = x_pool.tile([P, KT * n_cap], mybir.dt.bfloat16, name="X_sb", tag="xsb")
            # two SWDGE pieces along chunk-region boundaries
            half = len(chunks) - len(chunks) // 2
            for grp in (chunks[:half], chunks[half:]):
                if not grp:
                    continue
                a = KT * grp[0][0]
                bnd = KT * (grp[-1][0] + grp[-1][1])
                nc.gpsimd.dma_start(X_sb[:, a:bnd], Xd[:, a:bnd])

            for m in range(MT):
                ps = {}
                for ci in range(len(chunks)):
                    ps[ci] = ps_pool.tile([P, 512], mybir.dt.float32, name=f"ps{ci}", tag=f"ps{ci}")
                for k in range(KT):
                    kk = 0 if _FIXED_K_PROBE else k
                    wap = W_sb[:, m * D + kk * P : m * D + (kk + 1) * P]
                    for ci, (s, cs) in enumerate(chunks):
                        nc.tensor.matmul(
                            ps[ci][:, :cs],
                            wap,
                            X_sb[:, KT * s + k * cs : KT * s + (k + 1) * cs],
                            start=(k == 0),
                            stop=(k == KT - 1),
                            skip_group_check=True,
                        )
                o_sb = o_pool.tile([P, n_cap], mybir.dt.bfloat16, name="om", tag="om")
                for ci, (s, cs) in enumerate(chunks):
                    if (m + ci) % 2 == 0:
                        nc.scalar.copy(o_sb[:, s : s + cs], ps[ci][:, :cs])
                    else:
                        nc.vector.tensor_scalar_add(o_sb[:, s : s + cs], ps[ci][:, :cs], 0.0)
                nc.scalar.dma_start(Od[:, m * n_cap : (m + 1) * n_cap], o_sb[:])

    nc.compile()
    _dedup_ldweights(nc)
    _check_noload_pairs(nc)
    return nc


def _dedup_ldweights(nc):
    """tile_legalize emits one InstLdweights per bf16 matmul with no
    redundancy elision; back-to-back reloads of the already-loaded
    stationary AP cost ~53ns each (FWL) of pure PE stall. Delete every
    Ldweights whose AP is already loaded and that carries no semaphore
    wait/update (scheduling metadata only -- runtime order is unchanged)."""
    removed = 0
    for fn in nc.m.functions:
        for blk in fn.blocks:
            loaded = None
            keep = []
            n_before = len(blk.instructions)
            for inst in blk.instructions:
                tn = type(inst).__name__
                if tn == "InstLdweights":
                    key = str(inst.ins[0])
                    if key == loaded and not inst.has_wait() and not inst.has_update():
                        continue
                    loaded = key
                elif tn == "InstMatmult":
                    if inst.ldweights:
                        loaded = str(inst.ins[1])
                elif tn == "InstMatmultMx":
                    loaded = None
                keep.append(inst)
            if len(keep) != n_before:
                blk.instructions = keep
                removed += n_before - len(keep)
    return removed


def _check_noload_pairs(nc):
    """Every non-self-loading matmul must see the stationary operand its
    AP names actually loaded in the PE array -- for bf16, legalization
    emits a standalone InstLdweights before each InstMatmult, so track the
    last LDW (or self-loading matmul) and assert each matmul's stationary
    matches it. Scheduling is deterministic at build time, so passing here
    guarantees correctness on device."""
    loaded = None
    for fn in nc.m.functions:
        for blk in fn.blocks:
            for inst in blk.instructions:
                tn = type(inst).__name__
                if tn == "InstLdweights":
                    loaded = str(inst.ins[0])
                elif tn == "InstMatmult":
                    if inst.ldweights:
                        loaded = str(inst.ins[1])
                    else:
                        assert loaded is not None, "no-load matmul with nothing loaded"
                        assert str(inst.ins[1]) == loaded, (
                            f"no-load matmul stationary mismatch:\n"
                            f"loaded: {loaded}\nthis: {inst.ins[1]}"
                        )


class _Runner:
    """Caches the compiled NEFF + jitted shard_map executable for one n_cap."""

    def __init__(self, n_cap: int, reps: int = 1):
        self.n_cap = n_cap
        self.nc = _build(n_cap, reps)
        _b2j.install_neuronx_cc_hook()

        assert self.nc.dbg_addr is None
        partition_name = (
            self.nc.partition_id_tensor.name if self.nc.partition_id_tensor else None
        )

        in_names, out_names, out_avals = [], [], []
        for alloc in self.nc.m.functions[0].allocations:
            if not isinstance(alloc, mybir.MemoryLocationSet):
                continue
            name = alloc.memorylocations[0].name
            if alloc.kind == "ExternalInput":
                if name != partition_name:
                    in_names.append(name)
            elif alloc.kind == "ExternalOutput":
                out_names.append(name)
                out_avals.append(
                    jax.core.ShapedArray(
                        tuple(alloc.tensor_shape), mybir.dt.np(alloc.dtype)
                    )
                )
        self.in_names = in_names
        self.out_names = out_names
        self.out_avals = out_avals
        self.n_params = len(in_names)
        self.n_outs = len(out_names)
        all_in_names = tuple(in_names + out_names)
        if partition_name is not None:
            all_in_names = all_in_names + (partition_name,)

        nc = self.nc

        def _bind(*args):
            operands = list(args)
            if partition_name is not None:
                operands.append(_b2j.partition_id_tensor())
            return tuple(
                _b2j._bass_exec_p.bind(
                    *operands,
                    out_avals=tuple(out_avals),
                    in_names=all_in_names,
                    out_names=tuple(out_names),
                    lowering_input_output_aliases=(),
                    sim_require_finite=True,
                    sim_require_nnan=True,
                    nc=nc,
                )
            )

        self._bind = _bind
        self.devices = jax.devices("neuron")[:C]
        self.mesh = Mesh(np.asarray(self.devices), ("core",))
        spec_in = (PartitionSpec("core"),) * (self.n_params + self.n_outs)
        spec_out = (PartitionSpec("core"),) * self.n_outs
        self._spec_in, self._spec_out = spec_in, spec_out
        self._exec = jax.jit(
            shard_map(
                _bind,
                mesh=self.mesh,
                in_specs=spec_in,
                out_specs=spec_out,
                check_rep=False,
            ),
            donate_argnums=tuple(range(self.n_params, self.n_params + self.n_outs)),
            keep_unused=True,
        )

    def make_exec_nodonate(self):
        """Jitted executable that does not donate its output-init operands,
        so pre-staged device args can be reused across timing reps."""
        return jax.jit(
            shard_map(
                self._bind,
                mesh=self.mesh,
                in_specs=self._spec_in,
                out_specs=self._spec_out,
                check_rep=False,
            ),
            keep_unused=True,
        )

    def concat_inputs(self, in_maps):
        return [
            np.concatenate([np.asarray(m[name]) for m in in_maps], axis=0)
            for name in self.in_names
        ]

    def zero_outs(self):
        return [
            np.zeros((C * a.shape[0], *a.shape[1:]), a.dtype) for a in self.out_avals
        ]

    def run(self, in_maps):
        out_arrs = self._exec(*self.concat_inputs(in_maps), *self.zero_outs())
        return [
            {
                name: np.asarray(out_arrs[i]).reshape(C, *self.out_avals[i].shape)[c]
                for i, name in enumerate(self.out_names)
            }
            for c in range(C)
        ]


def _get(n_cap: int, reps: int = 1) -> _Runner:
    key = (n_cap, reps)
    if key not in _cache:
        _cache[key] = _Runner(n_cap, reps)
    return _cache[key]


def _prep(x, condition_ids, W, b):
    x = np.ascontiguousarray(np.asarray(x, dtype=np.float32))
    cond = np.asarray(condition_ids).astype(np.int64)
    W = np.asarray(W, dtype=np.float32)
    b = np.asarray(b, dtype=np.float32)

    bias_sum = b.sum(axis=0, dtype=np.float64).astype(np.float32)  # [D]

    rows = [np.nonzero(cond == c)[0] for c in range(C)]
    n_max = max(len(r) for r in rows)
    n_cap = max(16, -(-n_max // 16) * 16)
    chunks = _chunk_sizes(n_cap)

    in_maps = []
    for c in range(C):
        r = rows[c]
        xg = np.zeros((n_cap, D), np.float32)
        xg[: len(r)] = x[r]
        X3 = xg.astype(BF16).reshape(n_cap, KT, P)
        Ximg = np.empty((P, KT * n_cap), BF16)
        for s, cs in chunks:
            Ximg[:, KT * s : KT * (s + cs)] = (
                X3[s : s + cs].transpose(2, 1, 0).reshape(P, KT * cs)
            )
        # Wimg[p, m*D + k*P + j] = W[c][m*P + j, k*P + p]
        Wimg = np.ascontiguousarray(
            W[c].astype(BF16).reshape(MT, P, KT, P).transpose(3, 0, 2, 1).reshape(P, MT * D)
        )
        in_maps.append({"Ximg": Ximg, "Wimg": Wimg})
    return rows, n_cap, in_maps, bias_sum


def _postprocess(out_imgs, rows, n_cap, bias_sum):
    """out_imgs: per-core [P, MT*n_cap] bf16 -> full [B, D] fp32 output."""
    out = np.empty((B, D), np.float32)
    for c in range(C):
        r = rows[c]
        if len(r) == 0:
            continue
        O = np.asarray(out_imgs[c]).reshape(P, MT, n_cap)[:, :, : len(r)]
        res = O.transpose(2, 1, 0).reshape(len(r), D).astype(np.float32)
        out[r] = res + bias_sum
    return out


def _run(x, condition_ids, W, b):
    rows, n_cap, in_maps, bias_sum = _prep(x, condition_ids, W, b)
    runner = _get(n_cap)
    results = runner.run(in_maps)
    out = _postprocess([results[c]["Oimg"] for c in range(C)], rows, n_cap, bias_sum)
    return out, runner


def kernel(x, condition_ids, W, b):
    out, _ = _run(x, condition_ids, W, b)
    return out
